# revision 23
# baseline (speedup 1.0000x reference)
"""Trainium2 Bass kernel for nn_AtomicKANLayer (v5).

Math: y[b,o] = sum_{i,d} fupn((x[b,i]-centers[d])*compression[d]) * coeffs[i,o,d]
with fupn evaluated via its Fourier series.  Key structure exploited:

* The series coefficients decay superpolynomially: NK=5 terms leave 9.2e-4
  relative error (gate is 2e-2), so the trig features of EIGHT 512-element
  element-groups pack into 89 partitions (10 trig rows + 1 x row per group,
  one shared ones row) and every elementwise pass runs on a 512-wide free dim.
* With centers on the exact grid cent_d = d/16-1 and compression 16, the
  series argument z_d = u - d (u = 16x+16) is 3-PERIODIC in d: only three
  distinct series values S_c per element (c = d mod 3).  The per-(elem,d)
  basis is S_{d mod 3} * [z_d^2 <= a^2]; the d-replication of the three
  S_c values is folded into the ws matmul weights.
* The 2.1MB fp16 coefficient tensor is quantized to fp8e3 (e3m4, 1.06MB,
  measured end-to-end error 1.23e-2 vs the 2e-2 gate) and streamed over
  the sync+scalar HWDGE queues (the gpsimd SWDGE queue gets only the
  last-needed d-chunk: it wakes ~2us later and runs at half the rate).
* uqpq and ws are merged into ONE dram tensor so the critical small
  constants arrive in a single early burst on the sync queue, and the
  xh/ones trig rows are filled by an on-chip SBUF->SBUF copy instead of
  a separate HBM transfer.

The PE p-state governor only reaches full clock (~0.42ns/col) after
sustained full-array matmul power; known PE wait windows are filled with
full-K dummy matmuls into a scratch PSUM bank, chained by ordering-only
deps.  Too many warms trip the activity governor (50% duty cap).

Device pipeline per core (data-parallel over batch, 32 rows of B=256):
  1. q[p,c] = theta_p/2pi * x + off_p  (exact split-fp16 matmul, K=33)  [PE]
  2. f = q - round(q) via fp32 magic-constant (ts add/sub, tt sub)     [DVE]
  3. trig[0:80] = Sin(2pi f) -> fp16, two 256-col chunks               [ACT]
  4. S_j = trig-colslice.T @ w_s  -> [128 i, 8 groups x (z_d, S_d)],
     two matmuls per j into bank-aligned halves of a 2-bank PSUM tile  [PE]
  5. msq = z^2                                                         [ACT]
     bas = (msq <= a^2) * S_d                                          [DVE]
  6. y = sum_d bas_d.T @ co_d (33 fp16xfp8 matmuls, fp32 PSUM)          [PE]
  7. y = y_a + y_b -> DMA out (rows are (j,g)-permuted; host fixes)
"""
import sys

sys.path.insert(0, "/opt/trn_rl_repo")

import numpy as np

F16 = np.float16
B, I, O, D = 256, 128, 256, 33
NCORES = 8
BLOC = B // NCORES          # 32 batch rows per core
ROWS = BLOC * I             # 4096 flattened (b, i) elements per core
NG = 8                      # element groups packed along partitions
FD = ROWS // NG             # 512 free-dim columns per group
NK = 5                      # Fourier terms kept (of reference's 100)
GROWS = 2 * NK              # trig rows per group (cos+sin)
NPART = NG * GROWS + NG + 1  # 80 trig + 8 xh + 1 ones = 89
N_ORDER, NPROD = 1, 10
A_SUP = (N_ORDER + 2) / 2.0  # support half-width a = 1.5
CVAL = 16.0                  # compression (asserted on host)
MAGIC = float(np.float32(1.5 * 2**23))
TWO_PI = float(2 * np.pi)
QCH = 2                      # q -> sin chain chunks (of FD)
BAS_DIV = 256.0              # bas = basis/BAS_DIV (keeps fp16 bas normal)
CO_QMAX = 8.0                # quantize co to e3m4 with absmax -> 8.0
CO_FP8 = False               # fp16 co: the stream overlaps the bas chain
# co d-range chunks with ladder sizes per queue: within one queue the DGE
# round-robins packets across live transfers, so a chunk's completion sem
# fires at ~n_live * its own size into the stream -- small chunks first
# give early y gates.  sync gets d0-13 (after the consts burst), gpsimd
# (slow SWDGE) the mid range, scalar (wakes second, bursts) the tail.
CO_CH = [(0, 2), (2, 7), (7, 13),      # sync
         (13, 15), (15, 19),           # gpsimd
         (19, 22), (22, 27), (27, 33)]  # scalar
CO_ENG = [0, 0, 0, 2, 2, 1, 1, 1]      # 0=sync 1=scalar 2=gpsimd
PQC = FD + GROWS * NG        # consts col ranges: [0,FD)=u rows, [FD,PQC)=pq
CCOLS = PQC + 66 * NG        # [PQC,CCOLS) = ws
# PE warm-up dummies: (n_before_q, n_q_to_S, n_sin2_gap, n_S_to_y) 256-col
# full-K matmuls (the p-state governor ignores low-power work; too many
# trip the 50%-duty activity governor during the y contraction)
WARM = (8, 3, 1, 2)
DSPLIT = 16                  # y accumulator split point over d

_PROG = None


def _patch_act_tables(bacc):
    """Restrict the act-table chooser to the one set holding BOTH Sin and
    Square ('trig_and_small'), so a single 1.3us ACT_TABLE_LOAD is emitted
    instead of two.  Indices (act_func_set_ids) are preserved by keeping
    every set in place and just emptying the others."""
    import concourse.hw_specs as hw_specs
    real = hw_specs.get_activation_tables.__wrapped__

    def only_trig(module_arch):
        tabs = real(module_arch)
        return {name: (s if name == "trig_and_small" else set())
                for name, s in tabs.items()}

    bacc.get_activation_tables = only_trig


def _build_program():
    import concourse.bacc as bacc
    import concourse.tile as tile
    from concourse import mybir

    _patch_act_tables(bacc)

    f32 = mybir.dt.float32
    f16 = mybir.dt.float16
    codt = mybir.dt.float8e3 if CO_FP8 else f16
    Alu = mybir.AluOpType
    Act = mybir.ActivationFunctionType

    nc = bacc.Bacc("TRN2", target_bir_lowering=False, debug=False,
                   num_devices=NCORES)
    # +32 col pad: a fully-contiguous dram region coalesces onto a SINGLE
    # DMA engine (~24 B/ns); the pad keeps the transfer strided so its rows
    # spread across all 16 engines (~190 B/ns)
    cst_d = nc.dram_tensor("cst", [NPART, CCOLS + 32], f16,
                           kind="ExternalInput")
    co_d = nc.dram_tensor("co", [I, D * O], codt, kind="ExternalInput")
    y_d = nc.dram_tensor("y_s", [BLOC, O + 16], f32, kind="ExternalOutput")

    with tile.TileContext(nc) as tc:
        with (
            tc.tile_pool(name="const", bufs=1) as cpool,
            tc.tile_pool(name="work", bufs=2) as wpool,
            tc.tile_pool(name="sp", bufs=3, space="PSUM") as spool,
            tc.tile_pool(name="yp", bufs=1, space="PSUM") as ypool,
        ):
            # --- input DMAs.  The consts burst must own the sync queue
            # until it lands (in-queue transfers round-robin, so anything
            # pushed behind it delays its completion sem): the xh SBUF copy
            # naturally blocks the sync engine on the consts sem, and an
            # explicit engine-order chain keeps the scheduler from hoisting
            # the co pushes above it.  scalar pushes its co chunks before
            # its act-table load; gpsimd (SWDGE) carries the mid d range. ---
            from concourse.tile_rust import add_dep_helper
            scr = cpool.tile([128, FD], f16)     # warm-up operand
            nc.gpsimd.memset(scr[:], 0.0)
            # +32 col pad on the SBUF side too: a full-tile dst AP coalesces
            # into one descriptor chain served by a single DMA engine; a
            # column-slice dst splits per partition row across all 16
            cst = cpool.tile([NPART, CCOLS + 32], f16)
            trig = cpool.tile([NPART, FD], f16)
            co_t = cpool.tile([I, D * O], codt)

            se_last = [None]

            def se_order(instr):
                if se_last[0] is not None:
                    add_dep_helper(instr.ins, se_last[0].ins, sync=False,
                                   reason="sync queue order")
                se_last[0] = instr
                return instr

            se_order(nc.sync.dma_start(cst[:, 0:CCOLS],
                                       cst_d.ap()[:, 0:CCOLS]))
            # trig tile: rows 0..80 sin/cos features, 80..88 xh, 88 ones;
            # xh/ones rows are consts rows 0..9 (uh per group + ones)
            se_order(nc.sync.dma_start(trig[NG * GROWS:NPART, :],
                                       cst[0:NG + 1, 0:FD]))
            co_eng = [nc.sync, nc.scalar, nc.gpsimd]
            for ci, (d0, d1) in enumerate(CO_CH):
                instr = co_eng[CO_ENG[ci]].dma_start(
                    co_t[:, O * d0:O * d1], co_d.ap()[:, O * d0:O * d1])
                if CO_ENG[ci] == 0:
                    se_order(instr)

            # --- PE warm-up: keep the tensor engine busy (at full array
            # power) through every wait so the p-state governor ramps to and
            # holds full clock.  All PE instructions are chained with
            # ordering-only deps so the scheduler cannot interleave them.
            # Warms dump into y_a, which the first real y matmul resets. ---
            y_a = ypool.tile([BLOC, O], f32, tag="ya")
            y_b = ypool.tile([BLOC, O], f32, tag="yb")
            pe_last = [None]

            def pe_order(instr):
                if pe_last[0] is not None:
                    add_dep_helper(instr.ins, pe_last[0].ins, sync=False,
                                   reason="PE issue order")
                pe_last[0] = instr
                return instr

            def warm(n):
                for _ in range(n):
                    pe_order(nc.tensor.matmul(y_a[:], scr[:, 0:BLOC],
                                              scr[:, 0:O],
                                              start=True, stop=True))

            warm(WARM[0])

            # --- phase matmul + range reduction + Sin, 2 pipelined chunks.
            # PSUM banks: q chunks and S tiles share one 3-deep ring of
            # 2-bank tiles (q banks recycle into the later S tiles). ---
            cw = FD // QCH
            dve_last = [None]

            def dve_order(instr):
                if dve_last[0] is not None:
                    add_dep_helper(instr.ins, dve_last[0].ins, sync=False,
                                   reason="DVE chunk order")
                dve_last[0] = instr
                return instr

            for h in range(QCH):
                cs = slice(h * cw, (h + 1) * cw)
                q = spool.tile([128, 776], f32, tag="S")
                pe_order(nc.tensor.matmul(q[0:NG * GROWS, 0:cw],
                                          cst[0:33, FD:PQC],
                                          cst[0:33, cs],
                                          start=True, stop=True))
                f = wpool.tile([NG * GROWS, cw], f32, tag="f")
                qr = wpool.tile([NG * GROWS, cw], f32, tag="qr")
                dve_order(nc.vector.tensor_scalar(
                    qr[:], q[0:NG * GROWS, 0:cw], MAGIC, MAGIC,
                    op0=Alu.add, op1=Alu.subtract))
                dve_order(nc.vector.tensor_tensor(
                    f[:], q[0:NG * GROWS, 0:cw], qr[:],
                    op=Alu.subtract))
                nc.scalar.activation(trig[0:NG * GROWS, cs], f[:], Act.Sin,
                                     scale=TWO_PI)

            warm(WARM[1])

            # --- per 128-col slice j: series values + mask -> basis ---
            # S cols per group block of 66: [0..33) = z_d, [33..66) = S_d
            # bas col layout: m*D + d with m = 8j+g (host un-permutes rows)
            # matmul halves at cols 248/512 : bank-aligned dsts, adjacent
            # 528-col span so one Square/stt covers all 8 groups
            bas = cpool.tile([I, BLOC * D], f16)
            for j in range(4):
                if j == 2:
                    warm(WARM[2])
                S = spool.tile([128, 776], f32, tag="S")
                for h in range(2):
                    pe_order(nc.tensor.matmul(
                        S[:, 248 + 264 * h:248 + 264 * (h + 1)],
                        trig[:, 128 * j:128 * (j + 1)],
                        cst[:, PQC + 264 * h:PQC + 264 * h + 264],
                        start=True, stop=True))
                Sv = S[:, 248:776].rearrange("p (g c) -> p g c", c=66)
                msq = wpool.tile([128, NG * D], f32, tag="msq")
                nc.scalar.activation(
                    msq[:].rearrange("p (g d) -> p g d", d=D),
                    Sv[:, :, 0:D], Act.Square)
                bj = bas[:, NG * D * j:NG * D * (j + 1)].rearrange(
                    "p (g d) -> p g d", d=D)
                mv = msq[:].rearrange("p (g d) -> p g d", d=D)
                dve_order(nc.vector.scalar_tensor_tensor(
                    bj[:], in0=mv[:], scalar=A_SUP * A_SUP,
                    in1=Sv[:, :, D:2 * D], op0=Alu.is_le, op1=Alu.mult))

            warm(WARM[3])

            # --- final contraction over (i, d): two accumulators, d-chunk
            # waits injected by the tile framework per co slice ---
            basb = bas[:].rearrange("p (m c) -> p c m", c=D)
            for d in range(DSPLIT):
                pe_order(nc.tensor.matmul(y_a[:], basb[:, d, :],
                                          co_t[:, O * d:O * (d + 1)],
                                          start=(d == 0),
                                          stop=(d == DSPLIT - 1)))
            # y_a drains to SBUF while the y_b matmuls still run (only one
            # PSUM operand is allowed per DVE op anyway)
            y_sa = cpool.tile([BLOC, O], f32)
            nc.vector.tensor_scalar(y_sa[:], y_a[:], 1.0, None, op0=Alu.mult)
            for d in range(DSPLIT, D):
                pe_order(nc.tensor.matmul(y_b[:], basb[:, d, :],
                                          co_t[:, O * d:O * (d + 1)],
                                          start=(d == DSPLIT),
                                          stop=(d == D - 1)))
            y_s = cpool.tile([BLOC, O], f32)
            nc.vector.tensor_tensor(y_s[:], y_sa[:], y_b[:], op=Alu.add)
            nc.sync.dma_start(y_d.ap()[:, 0:O], y_s[:])

    nc.compile()
    return nc


def _fup_coeffs():
    k = np.arange(1, NK + 1, dtype=np.float64)
    t = (np.pi / A_SUP) * k
    sinc = lambda z: np.sinc(z / np.pi)
    c = sinc(t / 2.0) ** N_ORDER
    for j in range(1, NPROD + 1):
        c = c * sinc(t / (2.0 ** j))
    return t, c


def _host_constants(compression, centers):
    comp = np.asarray(compression, np.float64)
    cent = np.asarray(centers, np.float64)
    assert comp.shape == (D,) and cent.shape == (D,)
    assert np.all(comp == CVAL), "kernel assumes compression == 16"
    assert np.allclose(cent, np.arange(D) / 16.0 - 1.0, atol=0, rtol=0), \
        "kernel assumes centers on the d/16-1 grid"

    t, c = _fup_coeffs()

    # per-group phase constants theta/2pi split into fp16 hi+lo.
    # consts rows: 0..8 = uh[g]+ones, 9..17 = ul[g], 17..25 = uh2, 25..33 = ul2
    feat = np.concatenate([t * CVAL / (2 * np.pi)] * 2)   # cos rows, sin rows
    foff = np.concatenate([np.full(NK, 0.25), np.zeros(NK)])
    th_h = feat.astype(F16).astype(np.float64)
    th_l = (feat - th_h).astype(F16).astype(np.float64)
    pq = np.zeros((33, NG * GROWS), np.float64)
    for g in range(NG):
        ps = slice(GROWS * g, GROWS * (g + 1))
        pq[g, ps] = th_h
        pq[9 + g, ps] = th_h
        pq[17 + g, ps] = th_l
        pq[25 + g, ps] = th_l
    pq[8, :] = np.tile(foff, NG)

    # feature -> (z_0..32, S_0..32) weights; phi_kd = t_k*((d mod 3)-16)
    ws = np.zeros((NPART, 66 * NG), np.float64)
    for g in range(NG):
        gc = 66 * g
        for dd in range(D):                        # z_d = 16*xh + (16 - d)
            ws[NG * GROWS + g, gc + dd] = CVAL
            ws[NG * GROWS + NG, gc + dd] = CVAL - dd
            phi = t * ((dd % 3) - CVAL)
            sc = A_SUP * BAS_DIV     # 1/BAS_DIV folded into the S columns
            ws[GROWS * g:GROWS * g + NK, gc + D + dd] = c * np.cos(phi) / sc
            ws[GROWS * g + NK:GROWS * (g + 1), gc + D + dd] = \
                c * np.sin(phi) / sc
            ws[NG * GROWS + NG, gc + D + dd] = 0.5 / sc
    return pq.astype(F16), ws.astype(F16)


# row m of device output corresponds to batch row b = 4*(m%8) + m//8
_PERM = np.array([8 * (b % 4) + b // 4 for b in range(BLOC)])


def _run(inputs, trace=False, **kw):
    global _PROG
    from concourse.bass_utils import run_bass_kernel_spmd

    if _PROG is None:
        _PROG = _build_program()
    nc = _PROG

    x = np.ascontiguousarray(np.asarray(inputs["x"], np.float32))
    coeffs = np.asarray(inputs["atomic_coeffs"], np.float32)
    pq, ws = _host_constants(inputs["compression"], inputs["centers"])
    cot = coeffs.transpose(0, 2, 1).astype(np.float64)
    if CO_FP8:
        import ml_dtypes
        co_scale = CO_QMAX / np.abs(cot).max()
        co = np.ascontiguousarray(
            (cot * co_scale).astype(ml_dtypes.float8_e3m4).reshape(I, D * O))
    else:
        co_scale = BAS_DIV
        co = np.ascontiguousarray(
            (cot * co_scale).astype(F16).reshape(I, D * O))
    y_rescale = np.float32(BAS_DIV / co_scale)

    in_maps = []
    for cid in range(NCORES):
        xflat = x[cid * BLOC:(cid + 1) * BLOC].reshape(ROWS)
        uh = xflat.astype(F16)
        ul = (xflat - uh.astype(np.float32)).astype(F16)
        cst = np.zeros((NPART, CCOLS + 32), F16)
        for g in range(NG):
            es = slice(FD * g, FD * (g + 1))
            cst[g, 0:FD] = uh[es]
            cst[9 + g, 0:FD] = ul[es]
            cst[17 + g, 0:FD] = uh[es]
            cst[25 + g, 0:FD] = ul[es]
        cst[8, 0:FD] = F16(1.0)
        cst[0:33, FD:PQC] = pq
        cst[:, PQC:CCOLS] = ws
        in_maps.append({"cst": cst, "co": co})

    res = run_bass_kernel_spmd(nc, in_maps, core_ids=list(range(NCORES)),
                               trace=trace, **kw)
    y = np.concatenate(
        [res.results[c]["y_s"][:, 0:O][_PERM] for c in range(NCORES)], axis=0)
    return (y * y_rescale).astype(np.float32, copy=False), res


def kernel(**inputs):
    y, _ = _run(inputs, trace=False)
    return y


# revision 26
# speedup vs baseline: 1.2912x; 1.2912x over previous
"""Trainium2 Bass kernel for nn_AtomicKANLayer (v5).

Math: y[b,o] = sum_{i,d} fupn((x[b,i]-centers[d])*compression[d]) * coeffs[i,o,d]
with fupn evaluated via its Fourier series.  Key structure exploited:

* The series coefficients decay superpolynomially: NK=5 terms leave 9.2e-4
  relative error (gate is 2e-2), so the trig features of EIGHT 512-element
  element-groups pack into 89 partitions (10 trig rows + 1 x row per group,
  one shared ones row) and every elementwise pass runs on a 512-wide free dim.
* With centers on the exact grid cent_d = d/16-1 and compression 16, the
  series argument z_d = u - d (u = 16x+16) is 3-PERIODIC in d: only three
  distinct series values S_c per element (c = d mod 3).  The per-(elem,d)
  basis is S_{d mod 3} * [z_d^2 <= a^2]; the d-replication of the three
  S_c values is folded into the ws matmul weights.
* The 2.1MB fp16 coefficient tensor is quantized to fp8e3 (e3m4, 1.06MB,
  measured end-to-end error 1.23e-2 vs the 2e-2 gate) and streamed over
  the sync+scalar HWDGE queues (the gpsimd SWDGE queue gets only the
  last-needed d-chunk: it wakes ~2us later and runs at half the rate).
* uqpq and ws are merged into ONE dram tensor so the critical small
  constants arrive in a single early burst on the sync queue, and the
  xh/ones trig rows are filled by an on-chip SBUF->SBUF copy instead of
  a separate HBM transfer.

The PE p-state governor only reaches full clock (~0.42ns/col) after
sustained full-array matmul power; known PE wait windows are filled with
full-K dummy matmuls into a scratch PSUM bank, chained by ordering-only
deps.  Too many warms trip the activity governor (50% duty cap).

Device pipeline per core (data-parallel over batch, 32 rows of B=256):
  1. q[p,c] = theta_p/2pi * x + off_p  (exact split-fp16 matmul, K=33)  [PE]
  2. f = q - round(q) via fp32 magic-constant (ts add/sub, tt sub)     [DVE]
  3. trig[0:80] = Sin(2pi f) -> fp16, two 256-col chunks               [ACT]
  4. S_j = trig-colslice.T @ w_s  -> [128 i, 8 groups x (z_d, S_d)],
     two matmuls per j into bank-aligned halves of a 2-bank PSUM tile  [PE]
  5. msq = z^2                                                         [ACT]
     bas = (msq <= a^2) * S_d                                          [DVE]
  6. y = sum_d bas_d.T @ co_d (33 fp16xfp8 matmuls, fp32 PSUM)          [PE]
  7. y = y_a + y_b -> DMA out (rows are (j,g)-permuted; host fixes)
"""
import sys

sys.path.insert(0, "/opt/trn_rl_repo")

import numpy as np

F16 = np.float16
B, I, O, D = 256, 128, 256, 33
NCORES = 8
BLOC = B // NCORES          # 32 batch rows per core
ROWS = BLOC * I             # 4096 flattened (b, i) elements per core
NG = 8                      # element groups packed along partitions
FD = ROWS // NG             # 512 free-dim columns per group
NK = 5                      # Fourier terms kept (of reference's 100)
GROWS = 2 * NK              # trig rows per group (cos+sin)
NPART = NG * GROWS + NG + 1  # 80 trig + 8 xh + 1 ones = 89
N_ORDER, NPROD = 1, 10
A_SUP = (N_ORDER + 2) / 2.0  # support half-width a = 1.5
CVAL = 16.0                  # compression (asserted on host)
MAGIC = float(np.float32(1.5 * 2**23))
TWO_PI = float(2 * np.pi)
QCH = 2                      # q -> sin chain chunks (of FD)
BAS_DIV = 256.0              # bas = basis/BAS_DIV (keeps fp16 bas normal)
CO_QMAX = 8.0                # quantize co to e3m4 with absmax -> 8.0
CO_FP8 = False               # fp16 co: the stream overlaps the bas chain
# co d-range chunks with ladder sizes per queue: within one queue the DGE
# round-robins packets across live transfers, so a chunk's completion sem
# fires at ~n_live * its own size into the stream -- small chunks first
# give early y gates.  sync gets d0-13 (after the consts burst), gpsimd
# (slow SWDGE) the mid range, scalar (wakes second, bursts) the tail.
CO_CH = [(0, 2), (2, 7), (7, 13),      # sync
         (13, 15), (15, 19),           # gpsimd
         (19, 22), (22, 27), (27, 33)]  # scalar
CO_ENG = [0, 0, 0, 2, 2, 1, 1, 1]      # 0=sync 1=scalar 2=gpsimd
PQC = FD + GROWS * NG        # consts col ranges: [0,FD)=u rows, [FD,PQC)=pq
CCOLS = PQC + 66 * NG        # [PQC,CCOLS) = ws
# PE warm-up dummies: (n_before_q, n_q_to_S, n_sin2_gap, n_S_to_y) 256-col
# full-K matmuls (the p-state governor ignores low-power work; too many
# trip the 50%-duty activity governor during the y contraction)
WARM = (8, 3, 1, 2)
DSPLIT = 16                  # y accumulator split point over d

_PROG = None


def _patch_act_tables(bacc):
    """Restrict the act-table chooser to the one set holding BOTH Sin and
    Square ('trig_and_small'), so a single 1.3us ACT_TABLE_LOAD is emitted
    instead of two.  Indices (act_func_set_ids) are preserved by keeping
    every set in place and just emptying the others."""
    import concourse.hw_specs as hw_specs
    real = hw_specs.get_activation_tables.__wrapped__

    def only_trig(module_arch):
        tabs = real(module_arch)
        return {name: (s if name == "trig_and_small" else set())
                for name, s in tabs.items()}

    bacc.get_activation_tables = only_trig


def _build_program():
    import concourse.bacc as bacc
    import concourse.tile as tile
    from concourse import mybir

    _patch_act_tables(bacc)

    f32 = mybir.dt.float32
    f16 = mybir.dt.float16
    codt = mybir.dt.float8e3 if CO_FP8 else f16
    Alu = mybir.AluOpType
    Act = mybir.ActivationFunctionType

    nc = bacc.Bacc("TRN2", target_bir_lowering=False, debug=False,
                   num_devices=NCORES)
    # +32 col pad: a fully-contiguous dram region coalesces onto a SINGLE
    # DMA engine (~24 B/ns); the pad keeps the transfer strided so its rows
    # spread across all 16 engines (~190 B/ns)
    cst_d = nc.dram_tensor("cst", [128, CCOLS + 32], f16,
                           kind="ExternalInput")
    co_d = nc.dram_tensor("co", [I, D * O], codt, kind="ExternalInput")
    y_d = nc.dram_tensor("y_s", [BLOC, O + 16], f32, kind="ExternalOutput")

    with tile.TileContext(nc) as tc:
        with (
            tc.tile_pool(name="const", bufs=1) as cpool,
            tc.tile_pool(name="work", bufs=2) as wpool,
            tc.tile_pool(name="sp", bufs=3, space="PSUM") as spool,
            tc.tile_pool(name="yp", bufs=1, space="PSUM") as ypool,
        ):
            # --- input DMAs.  The consts burst must own the sync queue
            # until it lands (in-queue transfers round-robin, so anything
            # pushed behind it delays its completion sem): the xh SBUF copy
            # naturally blocks the sync engine on the consts sem, and an
            # explicit engine-order chain keeps the scheduler from hoisting
            # the co pushes above it.  scalar pushes its co chunks before
            # its act-table load; gpsimd (SWDGE) carries the mid d range. ---
            from concourse.tile_rust import add_dep_helper
            scr = cpool.tile([128, FD], f16)     # warm-up operand
            nc.gpsimd.memset(scr[:], 0.0)
            # +32 col pad on the SBUF side too: a full-tile dst AP coalesces
            # into one descriptor chain served by a single DMA engine; a
            # column-slice dst splits per partition row across all 16
            cst = cpool.tile([128, CCOLS + 32], f16)
            trig = cpool.tile([NPART, FD], f16)
            co_t = cpool.tile([I, D * O], codt)

            se_last = [None]

            def se_order(instr):
                if se_last[0] is not None:
                    add_dep_helper(instr.ins, se_last[0].ins, sync=False,
                                   reason="sync queue order")
                se_last[0] = instr
                return instr

            se_order(nc.sync.dma_start(cst[:, 0:CCOLS],
                                       cst_d.ap()[:, 0:CCOLS]))
            # trig tile: rows 0..80 sin/cos features, 80..88 xh, 88 ones;
            # xh/ones rows are consts rows 0..9 (uh per group + ones)
            se_order(nc.sync.dma_start(trig[NG * GROWS:NPART, :],
                                       cst[0:NG + 1, 0:FD]))
            co_eng = [nc.sync, nc.scalar, nc.gpsimd]
            for ci, (d0, d1) in enumerate(CO_CH):
                instr = co_eng[CO_ENG[ci]].dma_start(
                    co_t[:, O * d0:O * d1], co_d.ap()[:, O * d0:O * d1])
                if CO_ENG[ci] == 0:
                    se_order(instr)

            # --- PE warm-up: keep the tensor engine busy (at full array
            # power) through every wait so the p-state governor ramps to and
            # holds full clock.  All PE instructions are chained with
            # ordering-only deps so the scheduler cannot interleave them.
            # Warms dump into y_a, which the first real y matmul resets. ---
            y_a = ypool.tile([BLOC, O], f32, tag="ya")
            y_b = ypool.tile([BLOC, O], f32, tag="yb")
            pe_last = [None]

            def pe_order(instr):
                if pe_last[0] is not None:
                    add_dep_helper(instr.ins, pe_last[0].ins, sync=False,
                                   reason="PE issue order")
                pe_last[0] = instr
                return instr

            def warm(n):
                for _ in range(n):
                    pe_order(nc.tensor.matmul(y_a[:], scr[:, 0:BLOC],
                                              scr[:, 0:O],
                                              start=True, stop=True))

            warm(WARM[0])

            # --- phase matmul + range reduction + Sin, 2 pipelined chunks.
            # PSUM banks: q chunks and S tiles share one 3-deep ring of
            # 2-bank tiles (q banks recycle into the later S tiles). ---
            cw = FD // QCH
            dve_last = [None]

            def dve_order(instr):
                if dve_last[0] is not None:
                    add_dep_helper(instr.ins, dve_last[0].ins, sync=False,
                                   reason="DVE chunk order")
                dve_last[0] = instr
                return instr

            for h in range(QCH):
                cs = slice(h * cw, (h + 1) * cw)
                q = spool.tile([128, 776], f32, tag="S")
                pe_order(nc.tensor.matmul(q[0:NG * GROWS, 0:cw],
                                          cst[0:33, FD:PQC],
                                          cst[0:33, cs],
                                          start=True, stop=True))
                f = wpool.tile([NG * GROWS, cw], f32, tag="f")
                qr = wpool.tile([NG * GROWS, cw], f32, tag="qr")
                dve_order(nc.vector.tensor_scalar(
                    qr[:], q[0:NG * GROWS, 0:cw], MAGIC, MAGIC,
                    op0=Alu.add, op1=Alu.subtract))
                dve_order(nc.vector.tensor_tensor(
                    f[:], q[0:NG * GROWS, 0:cw], qr[:],
                    op=Alu.subtract))
                nc.scalar.activation(trig[0:NG * GROWS, cs], f[:], Act.Sin,
                                     scale=TWO_PI)

            warm(WARM[1])

            # --- per 128-col slice j: series values + mask -> basis ---
            # S cols per group block of 66: [0..33) = z_d, [33..66) = S_d
            # bas col layout: m*D + d with m = 8j+g (host un-permutes rows)
            # matmul halves at cols 248/512 : bank-aligned dsts, adjacent
            # 528-col span so one Square/stt covers all 8 groups
            bas = cpool.tile([I, BLOC * D], f16)
            for j in range(4):
                if j == 2:
                    warm(WARM[2])
                S = spool.tile([128, 776], f32, tag="S")
                for h in range(2):
                    pe_order(nc.tensor.matmul(
                        S[:, 248 + 264 * h:248 + 264 * (h + 1)],
                        trig[:, 128 * j:128 * (j + 1)],
                        cst[0:NPART, PQC + 264 * h:PQC + 264 * h + 264],
                        start=True, stop=True))
                Sv = S[:, 248:776].rearrange("p (g c) -> p g c", c=66)
                msq = wpool.tile([128, NG * D], f32, tag="msq")
                nc.scalar.activation(
                    msq[:].rearrange("p (g d) -> p g d", d=D),
                    Sv[:, :, 0:D], Act.Square)
                bj = bas[:, NG * D * j:NG * D * (j + 1)].rearrange(
                    "p (g d) -> p g d", d=D)
                mv = msq[:].rearrange("p (g d) -> p g d", d=D)
                dve_order(nc.vector.scalar_tensor_tensor(
                    bj[:], in0=mv[:], scalar=A_SUP * A_SUP,
                    in1=Sv[:, :, D:2 * D], op0=Alu.is_le, op1=Alu.mult))

            warm(WARM[3])

            # --- final contraction over (i, d): two accumulators, d-chunk
            # waits injected by the tile framework per co slice ---
            basb = bas[:].rearrange("p (m c) -> p c m", c=D)
            for d in range(DSPLIT):
                pe_order(nc.tensor.matmul(y_a[:], basb[:, d, :],
                                          co_t[:, O * d:O * (d + 1)],
                                          start=(d == 0),
                                          stop=(d == DSPLIT - 1)))
            # y_a drains to SBUF while the y_b matmuls still run (only one
            # PSUM operand is allowed per DVE op anyway)
            y_sa = cpool.tile([BLOC, O], f32)
            nc.vector.tensor_scalar(y_sa[:], y_a[:], 1.0, None, op0=Alu.mult)
            for d in range(DSPLIT, D):
                pe_order(nc.tensor.matmul(y_b[:], basb[:, d, :],
                                          co_t[:, O * d:O * (d + 1)],
                                          start=(d == DSPLIT),
                                          stop=(d == D - 1)))
            y_s = cpool.tile([BLOC, O], f32)
            nc.vector.tensor_tensor(y_s[:], y_sa[:], y_b[:], op=Alu.add)
            nc.sync.dma_start(y_d.ap()[:, 0:O], y_s[:])

    nc.compile()
    return nc


def _fup_coeffs():
    k = np.arange(1, NK + 1, dtype=np.float64)
    t = (np.pi / A_SUP) * k
    sinc = lambda z: np.sinc(z / np.pi)
    c = sinc(t / 2.0) ** N_ORDER
    for j in range(1, NPROD + 1):
        c = c * sinc(t / (2.0 ** j))
    return t, c


def _host_constants(compression, centers):
    comp = np.asarray(compression, np.float64)
    cent = np.asarray(centers, np.float64)
    assert comp.shape == (D,) and cent.shape == (D,)
    assert np.all(comp == CVAL), "kernel assumes compression == 16"
    assert np.allclose(cent, np.arange(D) / 16.0 - 1.0, atol=0, rtol=0), \
        "kernel assumes centers on the d/16-1 grid"

    t, c = _fup_coeffs()

    # per-group phase constants theta/2pi split into fp16 hi+lo.
    # consts rows: 0..8 = uh[g]+ones, 9..17 = ul[g], 17..25 = uh2, 25..33 = ul2
    feat = np.concatenate([t * CVAL / (2 * np.pi)] * 2)   # cos rows, sin rows
    foff = np.concatenate([np.full(NK, 0.25), np.zeros(NK)])
    th_h = feat.astype(F16).astype(np.float64)
    th_l = (feat - th_h).astype(F16).astype(np.float64)
    pq = np.zeros((33, NG * GROWS), np.float64)
    for g in range(NG):
        ps = slice(GROWS * g, GROWS * (g + 1))
        pq[g, ps] = th_h
        pq[9 + g, ps] = th_h
        pq[17 + g, ps] = th_l
        pq[25 + g, ps] = th_l
    pq[8, :] = np.tile(foff, NG)

    # feature -> (z_0..32, S_0..32) weights; phi_kd = t_k*((d mod 3)-16)
    ws = np.zeros((NPART, 66 * NG), np.float64)
    for g in range(NG):
        gc = 66 * g
        for dd in range(D):                        # z_d = 16*xh + (16 - d)
            ws[NG * GROWS + g, gc + dd] = CVAL
            ws[NG * GROWS + NG, gc + dd] = CVAL - dd
            phi = t * ((dd % 3) - CVAL)
            sc = A_SUP * BAS_DIV     # 1/BAS_DIV folded into the S columns
            ws[GROWS * g:GROWS * g + NK, gc + D + dd] = c * np.cos(phi) / sc
            ws[GROWS * g + NK:GROWS * (g + 1), gc + D + dd] = \
                c * np.sin(phi) / sc
            ws[NG * GROWS + NG, gc + D + dd] = 0.5 / sc
    return pq.astype(F16), ws.astype(F16)


# row m of device output corresponds to batch row b = 4*(m%8) + m//8
_PERM = np.array([8 * (b % 4) + b // 4 for b in range(BLOC)])


def _run(inputs, trace=False, **kw):
    global _PROG
    from concourse.bass_utils import run_bass_kernel_spmd

    if _PROG is None:
        _PROG = _build_program()
    nc = _PROG

    x = np.ascontiguousarray(np.asarray(inputs["x"], np.float32))
    coeffs = np.asarray(inputs["atomic_coeffs"], np.float32)
    pq, ws = _host_constants(inputs["compression"], inputs["centers"])
    cot = coeffs.transpose(0, 2, 1).astype(np.float64)
    if CO_FP8:
        import ml_dtypes
        co_scale = CO_QMAX / np.abs(cot).max()
        co = np.ascontiguousarray(
            (cot * co_scale).astype(ml_dtypes.float8_e3m4).reshape(I, D * O))
    else:
        co_scale = BAS_DIV
        co = np.ascontiguousarray(
            (cot * co_scale).astype(F16).reshape(I, D * O))
    y_rescale = np.float32(BAS_DIV / co_scale)

    in_maps = []
    for cid in range(NCORES):
        xflat = x[cid * BLOC:(cid + 1) * BLOC].reshape(ROWS)
        uh = xflat.astype(F16)
        ul = (xflat - uh.astype(np.float32)).astype(F16)
        cst = np.zeros((128, CCOLS + 32), F16)
        for g in range(NG):
            es = slice(FD * g, FD * (g + 1))
            cst[g, 0:FD] = uh[es]
            cst[9 + g, 0:FD] = ul[es]
            cst[17 + g, 0:FD] = uh[es]
            cst[25 + g, 0:FD] = ul[es]
        cst[8, 0:FD] = F16(1.0)
        cst[0:33, FD:PQC] = pq
        cst[0:NPART, PQC:CCOLS] = ws
        in_maps.append({"cst": cst, "co": co})

    res = run_bass_kernel_spmd(nc, in_maps, core_ids=list(range(NCORES)),
                               trace=trace, **kw)
    y = np.concatenate(
        [res.results[c]["y_s"][:, 0:O][_PERM] for c in range(NCORES)], axis=0)
    return (y * y_rescale).astype(np.float32, copy=False), res


def kernel(**inputs):
    y, _ = _run(inputs, trace=False)
    return y


# revision 34
# speedup vs baseline: 1.3173x; 1.0202x over previous
"""Trainium2 Bass kernel for nn_AtomicKANLayer (v5).

Math: y[b,o] = sum_{i,d} fupn((x[b,i]-centers[d])*compression[d]) * coeffs[i,o,d]
with fupn evaluated via its Fourier series.  Key structure exploited:

* The series coefficients decay superpolynomially: NK=5 terms leave 9.2e-4
  relative error (gate is 2e-2), so the trig features of EIGHT 512-element
  element-groups pack into 89 partitions (10 trig rows + 1 x row per group,
  one shared ones row) and every elementwise pass runs on a 512-wide free dim.
* With centers on the exact grid cent_d = d/16-1 and compression 16, the
  series argument z_d = u - d (u = 16x+16) is 3-PERIODIC in d: only three
  distinct series values S_c per element (c = d mod 3).  The per-(elem,d)
  basis is S_{d mod 3} * [z_d^2 <= a^2]; the d-replication of the three
  S_c values is folded into the ws matmul weights.
* The 2.1MB fp16 coefficient tensor is quantized to fp8e3 (e3m4, 1.06MB,
  measured end-to-end error 1.23e-2 vs the 2e-2 gate) and streamed over
  the sync+scalar HWDGE queues (the gpsimd SWDGE queue gets only the
  last-needed d-chunk: it wakes ~2us later and runs at half the rate).
* uqpq and ws are merged into ONE dram tensor so the critical small
  constants arrive in a single early burst on the sync queue, and the
  xh/ones trig rows are filled by an on-chip SBUF->SBUF copy instead of
  a separate HBM transfer.

The PE p-state governor only reaches full clock (~0.42ns/col) after
sustained full-array matmul power; known PE wait windows are filled with
full-K dummy matmuls into a scratch PSUM bank, chained by ordering-only
deps.  Too many warms trip the activity governor (50% duty cap).

Device pipeline per core (data-parallel over batch, 32 rows of B=256):
  1. q[p,c] = theta_p/2pi * x + off_p  (exact split-fp16 matmul, K=33)  [PE]
  2. f = q - round(q) via fp32 magic-constant (ts add/sub, tt sub)     [DVE]
  3. trig[0:80] = Sin(2pi f) -> fp16, two 256-col chunks               [ACT]
  4. S_j = trig-colslice.T @ w_s  -> [128 i, 8 groups x (z_d, S_d)],
     two matmuls per j into bank-aligned halves of a 2-bank PSUM tile  [PE]
  5. msq = z^2                                                         [ACT]
     bas = (msq <= a^2) * S_d                                          [DVE]
  6. y = sum_d bas_d.T @ co_d (33 fp16xfp8 matmuls, fp32 PSUM)          [PE]
  7. y = y_a + y_b -> DMA out (rows are (j,g)-permuted; host fixes)
"""
import sys

sys.path.insert(0, "/opt/trn_rl_repo")

import numpy as np

F16 = np.float16
B, I, O, D = 256, 128, 256, 33
NCORES = 8
BLOC = B // NCORES          # 32 batch rows per core
ROWS = BLOC * I             # 4096 flattened (b, i) elements per core
NG = 8                      # element groups packed along partitions
FD = ROWS // NG             # 512 free-dim columns per group
NK = 5                      # Fourier terms kept (of reference's 100)
GROWS = 2 * NK              # trig rows per group (cos+sin)
NPART = NG * GROWS + NG + 1  # 80 trig + 8 xh + 1 ones = 89
N_ORDER, NPROD = 1, 10
A_SUP = (N_ORDER + 2) / 2.0  # support half-width a = 1.5
CVAL = 16.0                  # compression (asserted on host)
MAGIC = float(np.float32(1.5 * 2**23))
TWO_PI = float(2 * np.pi)
QCH = 2                      # q -> sin chain chunks (of FD)
BAS_DIV = 256.0              # bas = basis/BAS_DIV (keeps fp16 bas normal)
CO_QMAX = 8.0                # quantize co to e3m4 with absmax -> 8.0
CO_FP8 = False               # fp16 co: the stream overlaps the bas chain
# co d-range chunks with ladder sizes per queue: within one queue the DGE
# round-robins packets across live transfers, so a chunk's completion sem
# fires at ~n_live * its own size into the stream -- small chunks first
# give early y gates.  sync gets d0-13 (after the consts burst), gpsimd
# (slow SWDGE) the mid range, scalar (wakes second, bursts) the tail.
CO_CH = [(0, 2), (2, 6),               # gpsimd (early d, wakes late)
         (6, 10), (10, 16), (16, 22),  # scalar (mid d)
         (22, 26), (26, 33)]           # sync (late d; queue busy with cst)
CO_ENG = [2, 2, 1, 1, 1, 0, 0]         # 0=sync 1=scalar 2=gpsimd
PQC = FD + GROWS * NG        # consts col ranges: [0,FD)=u rows, [FD,PQC)=pq
WS2 = PQC + 66 * NG          # [PQC,WS2) = ws trig rows (partitions 0:80)
CCOLS = WS2 + 66 * NG        # [WS2,CCOLS) = ws xh/ones rows (partitions 0:9)
# PE warm-up dummies: (n_before_q, n_q_to_S, n_sin2_gap, n_S_to_y) 256-col
# full-K matmuls (the p-state governor ignores low-power work; too many
# trip the 50%-duty activity governor during the y contraction)
WARM = (4, 2, 1, 6)
DSPLIT = 16                  # y accumulator split point over d

_PROG = None


def _patch_act_tables(bacc):
    """Restrict the act-table chooser to the one set holding BOTH Sin and
    Square ('trig_and_small'), so a single 1.3us ACT_TABLE_LOAD is emitted
    instead of two.  Indices (act_func_set_ids) are preserved by keeping
    every set in place and just emptying the others."""
    import concourse.hw_specs as hw_specs
    real = hw_specs.get_activation_tables.__wrapped__

    def only_trig(module_arch):
        tabs = real(module_arch)
        return {name: (s if name == "trig_and_small" else set())
                for name, s in tabs.items()}

    bacc.get_activation_tables = only_trig


def _build_program():
    import concourse.bacc as bacc
    import concourse.tile as tile
    from concourse import mybir

    _patch_act_tables(bacc)

    f32 = mybir.dt.float32
    f16 = mybir.dt.float16
    codt = mybir.dt.float8e3 if CO_FP8 else f16
    Alu = mybir.AluOpType
    Act = mybir.ActivationFunctionType

    nc = bacc.Bacc("TRN2", target_bir_lowering=False, debug=False,
                   num_devices=NCORES)
    # +32 col pad: a fully-contiguous dram region coalesces onto a SINGLE
    # DMA engine (~24 B/ns); the pad keeps the transfer strided so its rows
    # spread across all 16 engines (~190 B/ns)
    cst_d = nc.dram_tensor("cst", [128, CCOLS + 32], f16,
                           kind="ExternalInput")
    co_d = nc.dram_tensor("co", [I, D * O], codt, kind="ExternalInput")
    y_d = nc.dram_tensor("y_s", [BLOC, O + 16], f32, kind="ExternalOutput")

    with tile.TileContext(nc) as tc:
        with (
            tc.tile_pool(name="const", bufs=1) as cpool,
            tc.tile_pool(name="work", bufs=2) as wpool,
            tc.tile_pool(name="sp", bufs=3, space="PSUM") as spool,
            tc.tile_pool(name="yp", bufs=1, space="PSUM") as ypool,
        ):
            # --- input DMAs.  The consts burst must own the sync queue
            # until it lands (in-queue transfers round-robin, so anything
            # pushed behind it delays its completion sem): the xh SBUF copy
            # naturally blocks the sync engine on the consts sem, and an
            # explicit engine-order chain keeps the scheduler from hoisting
            # the co pushes above it.  scalar pushes its co chunks before
            # its act-table load; gpsimd (SWDGE) carries the mid d range. ---
            from concourse.tile_rust import add_dep_helper
            scr = cpool.tile([128, FD], f16)     # warm-up operand
            nc.gpsimd.memset(scr[:], 0.0)
            # +32 col pad on the SBUF side too: a full-tile dst AP coalesces
            # into one descriptor chain served by a single DMA engine; a
            # column-slice dst splits per partition row across all 16
            cst = cpool.tile([128, CCOLS + 32], f16)
            trig = cpool.tile([NG * GROWS, FD], f16)
            co_t = cpool.tile([I, D * O], codt)

            nc.sync.dma_start(cst[:, 0:CCOLS], cst_d.ap()[:, 0:CCOLS])
            co_eng = [nc.sync, nc.scalar, nc.gpsimd]
            for ci, (d0, d1) in enumerate(CO_CH):
                if CO_ENG[ci] == 0:
                    continue                     # sync chunks pushed later
                co_eng[CO_ENG[ci]].dma_start(
                    co_t[:, O * d0:O * d1], co_d.ap()[:, O * d0:O * d1])

            # --- PE warm-up: keep the tensor engine busy (at full array
            # power) through every wait so the p-state governor ramps to and
            # holds full clock.  All PE instructions are chained with
            # ordering-only deps so the scheduler cannot interleave them.
            # Warms dump into y_a, which the first real y matmul resets. ---
            y_a = ypool.tile([BLOC, O], f32, tag="ya")
            y_b = ypool.tile([BLOC, O], f32, tag="yb")
            pe_last = [None]

            def pe_order(instr):
                if pe_last[0] is not None:
                    add_dep_helper(instr.ins, pe_last[0].ins, sync=False,
                                   reason="PE issue order")
                pe_last[0] = instr
                return instr

            def warm(n):
                for _ in range(n):
                    pe_order(nc.tensor.matmul(y_a[:], scr[:, 0:BLOC],
                                              scr[:, 0:O],
                                              start=True, stop=True))

            warm(WARM[0])

            # --- phase matmul + range reduction + Sin, 2 pipelined chunks.
            # PSUM banks: q chunks and S tiles share one 3-deep ring of
            # 2-bank tiles (q banks recycle into the later S tiles). ---
            cw = FD // QCH
            dve_last = [None]

            def dve_order(instr):
                if dve_last[0] is not None:
                    add_dep_helper(instr.ins, dve_last[0].ins, sync=False,
                                   reason="DVE chunk order")
                dve_last[0] = instr
                return instr

            qmm = []
            for h in range(QCH):
                cs = slice(h * cw, (h + 1) * cw)
                q = spool.tile([128, 776], f32, tag="S")
                qmm.append(pe_order(nc.tensor.matmul(
                    q[0:NG * GROWS, 0:cw], cst[0:33, FD:PQC], cst[0:33, cs],
                    start=True, stop=True)))
                f = wpool.tile([NG * GROWS, cw], f32, tag="f")
                qr = wpool.tile([NG * GROWS, cw], f32, tag="qr")
                dve_order(nc.vector.tensor_scalar(
                    qr[:], q[0:NG * GROWS, 0:cw], MAGIC, MAGIC,
                    op0=Alu.add, op1=Alu.subtract))
                dve_order(nc.vector.tensor_tensor(
                    f[:], q[0:NG * GROWS, 0:cw], qr[:],
                    op=Alu.subtract))
                nc.scalar.activation(trig[0:NG * GROWS, cs], f[:], Act.Sin,
                                     scale=TWO_PI)

            # sync's co pushes ride behind the consts burst: a synced dep on
            # the first q matmul (which itself waits the consts sem) keeps
            # their descriptors out of the queue until consts has drained,
            # without blocking the sync engine on a DMA sem directly
            for ci, (d0, d1) in enumerate(CO_CH):
                if CO_ENG[ci] != 0:
                    continue
                instr = nc.sync.dma_start(
                    co_t[:, O * d0:O * d1], co_d.ap()[:, O * d0:O * d1])
                add_dep_helper(instr.ins, qmm[0].ins, sync=True,
                               reason="co behind consts")

            warm(WARM[1])

            # --- per 128-col slice j: series values + mask -> basis ---
            # S cols per group block of 66: [0..33) = z_d, [33..66) = S_d
            # bas col layout: m*D + d with m = 8j+g (host un-permutes rows)
            # matmul halves at cols 248/512 : bank-aligned dsts, adjacent
            # 528-col span so one Square/stt covers all 8 groups
            bas = cpool.tile([I, BLOC * D], f16)
            for j in range(4):
                if j == 2:
                    warm(WARM[2])
                S = spool.tile([128, 776], f32, tag="S")
                for h in range(2):
                    # K-split accumulation: trig rows (K=80) + the xh/ones
                    # rows straight from the consts u-region (K=9) -- no
                    # on-chip copy of the xh rows into the trig tile needed
                    pe_order(nc.tensor.matmul(
                        S[:, 248 + 264 * h:248 + 264 * (h + 1)],
                        trig[:, 128 * j:128 * (j + 1)],
                        cst[0:NG * GROWS, PQC + 264 * h:PQC + 264 * h + 264],
                        start=True, stop=False))
                    pe_order(nc.tensor.matmul(
                        S[:, 248 + 264 * h:248 + 264 * (h + 1)],
                        cst[0:NG + 1, 128 * j:128 * (j + 1)],
                        cst[0:NG + 1, WS2 + 264 * h:WS2 + 264 * h + 264],
                        start=False, stop=True))
                Sv = S[:, 248:776].rearrange("p (g c) -> p g c", c=66)
                msq = wpool.tile([128, NG * D], f32, tag="msq")
                nc.scalar.activation(
                    msq[:].rearrange("p (g d) -> p g d", d=D),
                    Sv[:, :, 0:D], Act.Square)
                bj = bas[:, NG * D * j:NG * D * (j + 1)].rearrange(
                    "p (g d) -> p g d", d=D)
                mv = msq[:].rearrange("p (g d) -> p g d", d=D)
                dve_order(nc.vector.scalar_tensor_tensor(
                    bj[:], in0=mv[:], scalar=A_SUP * A_SUP,
                    in1=Sv[:, :, D:2 * D], op0=Alu.is_le, op1=Alu.mult))

            warm(WARM[3])

            # --- final contraction over (i, d): two accumulators, d-chunk
            # waits injected by the tile framework per co slice ---
            basb = bas[:].rearrange("p (m c) -> p c m", c=D)
            for d in range(DSPLIT):
                pe_order(nc.tensor.matmul(y_a[:], basb[:, d, :],
                                          co_t[:, O * d:O * (d + 1)],
                                          start=(d == 0),
                                          stop=(d == DSPLIT - 1)))
            # y_a drains to SBUF while the y_b matmuls still run (only one
            # PSUM operand is allowed per DVE op anyway)
            y_sa = cpool.tile([BLOC, O], f32)
            nc.vector.tensor_scalar(y_sa[:], y_a[:], 1.0, None, op0=Alu.mult)
            for d in range(DSPLIT, D):
                pe_order(nc.tensor.matmul(y_b[:], basb[:, d, :],
                                          co_t[:, O * d:O * (d + 1)],
                                          start=(d == DSPLIT),
                                          stop=(d == D - 1)))
            y_s = cpool.tile([BLOC, O], f32)
            nc.vector.tensor_tensor(y_s[:], y_sa[:], y_b[:], op=Alu.add)
            nc.sync.dma_start(y_d.ap()[:, 0:O], y_s[:])

    nc.compile()
    return nc


def _fup_coeffs():
    k = np.arange(1, NK + 1, dtype=np.float64)
    t = (np.pi / A_SUP) * k
    sinc = lambda z: np.sinc(z / np.pi)
    c = sinc(t / 2.0) ** N_ORDER
    for j in range(1, NPROD + 1):
        c = c * sinc(t / (2.0 ** j))
    return t, c


def _host_constants(compression, centers):
    comp = np.asarray(compression, np.float64)
    cent = np.asarray(centers, np.float64)
    assert comp.shape == (D,) and cent.shape == (D,)
    assert np.all(comp == CVAL), "kernel assumes compression == 16"
    assert np.allclose(cent, np.arange(D) / 16.0 - 1.0, atol=0, rtol=0), \
        "kernel assumes centers on the d/16-1 grid"

    t, c = _fup_coeffs()

    # per-group phase constants theta/2pi split into fp16 hi+lo.
    # consts rows: 0..8 = uh[g]+ones, 9..17 = ul[g], 17..25 = uh2, 25..33 = ul2
    feat = np.concatenate([t * CVAL / (2 * np.pi)] * 2)   # cos rows, sin rows
    foff = np.concatenate([np.full(NK, 0.25), np.zeros(NK)])
    th_h = feat.astype(F16).astype(np.float64)
    th_l = (feat - th_h).astype(F16).astype(np.float64)
    pq = np.zeros((33, NG * GROWS), np.float64)
    for g in range(NG):
        ps = slice(GROWS * g, GROWS * (g + 1))
        pq[g, ps] = th_h
        pq[9 + g, ps] = th_h
        pq[17 + g, ps] = th_l
        pq[25 + g, ps] = th_l
    pq[8, :] = np.tile(foff, NG)

    # feature -> (z_0..32, S_0..32) weights; phi_kd = t_k*((d mod 3)-16)
    ws = np.zeros((NPART, 66 * NG), np.float64)
    for g in range(NG):
        gc = 66 * g
        for dd in range(D):                        # z_d = 16*xh + (16 - d)
            ws[NG * GROWS + g, gc + dd] = CVAL
            ws[NG * GROWS + NG, gc + dd] = CVAL - dd
            phi = t * ((dd % 3) - CVAL)
            sc = A_SUP * BAS_DIV     # 1/BAS_DIV folded into the S columns
            ws[GROWS * g:GROWS * g + NK, gc + D + dd] = c * np.cos(phi) / sc
            ws[GROWS * g + NK:GROWS * (g + 1), gc + D + dd] = \
                c * np.sin(phi) / sc
            ws[NG * GROWS + NG, gc + D + dd] = 0.5 / sc
    return pq.astype(F16), ws.astype(F16)


# row m of device output corresponds to batch row b = 4*(m%8) + m//8
_PERM = np.array([8 * (b % 4) + b // 4 for b in range(BLOC)])


def _run(inputs, trace=False, **kw):
    global _PROG
    from concourse.bass_utils import run_bass_kernel_spmd

    if _PROG is None:
        _PROG = _build_program()
    nc = _PROG

    x = np.ascontiguousarray(np.asarray(inputs["x"], np.float32))
    coeffs = np.asarray(inputs["atomic_coeffs"], np.float32)
    pq, ws = _host_constants(inputs["compression"], inputs["centers"])
    cot = coeffs.transpose(0, 2, 1).astype(np.float64)
    if CO_FP8:
        import ml_dtypes
        co_scale = CO_QMAX / np.abs(cot).max()
        co = np.ascontiguousarray(
            (cot * co_scale).astype(ml_dtypes.float8_e3m4).reshape(I, D * O))
    else:
        co_scale = BAS_DIV
        co = np.ascontiguousarray(
            (cot * co_scale).astype(F16).reshape(I, D * O))
    y_rescale = np.float32(BAS_DIV / co_scale)

    in_maps = []
    for cid in range(NCORES):
        xflat = x[cid * BLOC:(cid + 1) * BLOC].reshape(ROWS)
        uh = xflat.astype(F16)
        ul = (xflat - uh.astype(np.float32)).astype(F16)
        cst = np.zeros((128, CCOLS + 32), F16)
        for g in range(NG):
            es = slice(FD * g, FD * (g + 1))
            cst[g, 0:FD] = uh[es]
            cst[9 + g, 0:FD] = ul[es]
            cst[17 + g, 0:FD] = uh[es]
            cst[25 + g, 0:FD] = ul[es]
        cst[8, 0:FD] = F16(1.0)
        cst[0:33, FD:PQC] = pq
        cst[0:NG * GROWS, PQC:WS2] = ws[0:NG * GROWS]
        cst[0:NG + 1, WS2:CCOLS] = ws[NG * GROWS:NPART]
        in_maps.append({"cst": cst, "co": co})

    res = run_bass_kernel_spmd(nc, in_maps, core_ids=list(range(NCORES)),
                               trace=trace, **kw)
    y = np.concatenate(
        [res.results[c]["y_s"][:, 0:O][_PERM] for c in range(NCORES)], axis=0)
    return (y * y_rescale).astype(np.float32, copy=False), res


def kernel(**inputs):
    y, _ = _run(inputs, trace=False)
    return y
